# revision 1
# baseline (speedup 1.0000x reference)
"""D2Q9 Lattice-Boltzmann single step (collide + stream + bounce-back + lift)
on 8 Trainium2 NeuronCores — planar-channel redesign.

Contract: kernel(**inputs) takes FULL inputs
  f [2048,2048,9] f32, rho [2048,2048] f32, u [2048,2048,2] f32,
  obstacle_mask [2048,2048] bool
and returns the FULL output [2048,2048,12] f32
  (f_new[9], rho_new, u_new[2] packed on the last axis).

Sharding: 1-D domain decomposition over rows; each core gets 256 rows plus
1-row/1-col wraparound halos (host-built). Host packs 13 bf16 PLANES per
core: F_i = (1-1/tau)*f_i (9), R = (1/(9 tau))*rho, XB = 3 ux, YB = 3 uy,
M = mask. All remaining math runs on device:

  B1 = R*XB   B2 = R*YB   D1 = B1*XB  D2 = B2*YB  X9 = B1*YB  (Pool stt)
  s = D1+D2   P = R - s/6 A1 = P+D1/2 A2 = P+D2/2 Q = P+s/2   (Pool stt)
  u = B1+B2 | v = B1-B2  Qx5 = Q+X9  Qx6 = Q-X9               (Pool/DVE)
  f*_i = F_i + feq'_i via PSUM-accumulated TensorE matmuls with scaled
  identity weights (feq'_i linear in {P,A1,A2,Qx5,Qx6,B1,B2,u,v}).
  Stream: row-shift via shifted-identity matmul, col-shift via AP offset.
  Bounce-back: copy_predicated on DVE. Lift: rho via 9 accumulated identity
  matmuls (PSUM), m1/m2 trees + reciprocal + mults on DVE.

Engines are load-balanced: Pool (gpsimd) runs the moment products as
scalar_tensor_tensor, PE runs all linear combinations, ACT drains PSUM,
DVE keeps only bounce-back/lift leftovers. Output is 12 bf16 planes,
reassembled/interleaved on host."""

import numpy as np
import concourse.bass as bass
import concourse.bacc as bacc
import concourse.mybir as mybir
from concourse import tile
from concourse.bass_utils import run_bass_kernel_spmd

NX = 2048
NY = 2048
NCORES = 8
R = NX // NCORES          # 256 rows per core
SLAB = R + 2              # 258 rows incl halos
YP = NY + 2               # 2050 cols incl halos

TAU = 0.6
IT = 1.0 / TAU            # 5/3
FCOEF = 1.0 - IT          # -2/3

EX = [0, 1, 0, -1, 0, 1, -1, -1, 1]
EY = [0, 0, 1, 0, -1, 1, 1, -1, -1]
OPP = [0, 3, 4, 1, 2, 7, 8, 5, 6]

W = 512                   # y-chunk width (output cols per chunk)
FW = W + 2                # 1026, chunk width incl y-halos
NPASS = 3                 # fs assembly psum passes
PW = FW // NPASS          # 342 cols per pass
NCH = 12                  # input planes (mask is a separate u8 tensor)
NCO = 12                  # output planes
TB = [0, 130]             # row-tile bases (slab rows tb..tb+127)

FP32 = mybir.dt.float32
BF16 = mybir.dt.bfloat16
U8 = mybir.dt.uint8
AL = mybir.AluOpType

# weight matrix column offsets in shm [128, 992]
C_I = 0       # identity
C_4I = 128    # 4*I
C_NI = 256    # -I
C_QI = 384    # 0.25*I
C_NQI = 512   # -0.25*I
C_SP = 640    # shift ex=+1 (out m = in m-1)
C_SM = 768    # shift ex=-1 (out m = in m+1)
C_PX = {1: 896, 0: 928, -1: 960}   # fixup perms [48 -> 32]
SHM_COLS = 992

# fixup geometry: out rows 126..129 <- slab rows 127..130, sources 126..131
FX_R0 = 126               # first slab row loaded
FX_NR = 6                 # slab rows per segment
FX_SEG = 8                # y segments
FX_W = NY // FX_SEG       # 256 out cols per segment
FX_F = FX_W + 2           # 258 incl halos
FX_PI = FX_SEG * FX_NR    # 48 input partitions
FX_NO = 4                 # out rows per segment
FX_PO = FX_SEG * FX_NO    # 32 output partitions

# fs-assembly spec: dir -> [(weight col, plane name), ...]
ASPEC = {
    0: [(C_I, 'F0'), (C_4I, 'P')],
    1: [(C_I, 'F1'), (C_I, 'A1'), (C_I, 'B1')],
    2: [(C_I, 'F2'), (C_I, 'A2'), (C_I, 'B2')],
    3: [(C_I, 'F3'), (C_I, 'A1'), (C_NI, 'B1')],
    4: [(C_I, 'F4'), (C_I, 'A2'), (C_NI, 'B2')],
    5: [(C_I, 'F5'), (C_QI, 'Q'), (C_QI, 'X9'), (C_QI, 'u')],
    6: [(C_I, 'F6'), (C_QI, 'Q'), (C_NQI, 'X9'), (C_NQI, 'v')],
    7: [(C_I, 'F7'), (C_QI, 'Q'), (C_QI, 'X9'), (C_NQI, 'u')],
    8: [(C_I, 'F8'), (C_QI, 'Q'), (C_NQI, 'X9'), (C_QI, 'v')],
}


def _moments(nc, scr, P, FW_, inview):
    """Moment/helper planes from the 13 input plane views.
    Pool: plain products/adds (SBUF only). DVE: scaled chain ops."""
    gp = nc.gpsimd
    vec = nc.vector

    def t(name):
        return scr.tile([P, FW_], BF16, tag=name, name=name)[:]

    Rv = inview(9)
    XB = inview(10)
    YB = inview(11)
    pl = {}
    for i in range(9):
        pl[f'F{i}'] = inview(i)
    B1 = t("B1"); gp.tensor_tensor(B1, Rv, XB, AL.mult)
    B2 = t("B2"); vec.tensor_tensor(B2, Rv, YB, AL.mult)
    D1 = t("D1"); gp.tensor_tensor(D1, B1, XB, AL.mult)
    D2 = t("D2"); vec.tensor_tensor(D2, B2, YB, AL.mult)
    X9 = t("X9"); gp.tensor_tensor(X9, B1, YB, AL.mult)
    h2 = t("ts2"); vec.tensor_scalar_mul(h2, D2, 0.5)
    s = t("s");   vec.tensor_tensor(s, D1, D2, AL.add)
    su = t("ts1"); vec.tensor_scalar_mul(su, s, 1.0 / 6.0)
    Pv = t("P");  vec.tensor_tensor(Pv, Rv, su, AL.subtract)
    h1 = su; vec.tensor_scalar_mul(h1, D1, 0.5)
    A1 = t("A1"); vec.tensor_tensor(A1, Pv, h1, AL.add)
    A2 = t("A2"); vec.tensor_tensor(A2, Pv, h2, AL.add)
    sh = h2; vec.tensor_scalar_mul(sh, s, 0.5)
    Q = t("Q");   vec.tensor_tensor(Q, Pv, sh, AL.add)
    uv = t("u");  gp.tensor_tensor(uv, B1, B2, AL.add)
    vv = t("v");  gp.tensor_tensor(vv, B1, B2, AL.subtract)
    pl.update(P=Pv, B1=B1, B2=B2, A1=A1, A2=A2, Q=Q, X9=X9, u=uv, v=vv)
    return pl


def _assemble_fs(nc, psA, shm, P, FW_, passes, pl, fsv, nametag):
    """PE: fs planes (f*) from basis planes via accumulated matmuls.
    fsv(i) -> [P, FW_] bf16 dest view (plane i of the fs tile)."""
    for p, (pc0, pw) in enumerate(passes):
        cs = slice(pc0, pc0 + pw)
        for i in range(9):
            ps = psA.tile([P, pw], FP32, tag="fsP", name=f"{nametag}fs{i}p{p}")
            terms = ASPEC[i]
            for k, (wc, pn) in enumerate(terms):
                nc.tensor.matmul(ps[:], shm[0:P, wc:wc + P], pl[pn][:, cs],
                                 start=(k == 0), stop=(k == len(terms) - 1))
            nc.scalar.copy(fsv(i)[:, cs], ps[:])


def _lift_finish(nc, scr, P, Wd, ov, inv, nametag):
    """m1/m2 trees + u = m/rho on DVE. ov(i) -> [P, Wd] out plane view,
    inv: [P, Wd] f32 AP (1/rho)."""
    vec = nc.vector

    def t(name, dt=BF16):
        return scr.tile([P, Wd], dt, tag=name, name="l" + name)[:]

    gp = nc.gpsimd
    d1 = t("d1"); gp.tensor_tensor(d1, ov(1), ov(3), AL.subtract)
    d5 = t("d5"); gp.tensor_tensor(d5, ov(5), ov(7), AL.subtract)
    d8 = t("d8"); gp.tensor_tensor(d8, ov(8), ov(6), AL.subtract)
    e1 = d1; gp.tensor_tensor(e1, d1, d5, AL.add)
    m1 = scr.tile([P, Wd], FP32, tag="m1", name="m1")[:]
    m2 = scr.tile([P, Wd], FP32, tag="m2", name="m2")[:]
    vec.tensor_tensor(m1, e1, d8, AL.add)
    d2 = t("d2"); gp.tensor_tensor(d2, ov(2), ov(4), AL.subtract)
    e2 = d2; gp.tensor_tensor(e2, d2, d5, AL.add)
    vec.tensor_tensor(m2, e2, d8, AL.subtract)
    vec.tensor_tensor(ov(10), m1, inv, AL.mult)
    vec.tensor_tensor(ov(11), m2, inv, AL.mult)


def _build_program():
    nc = bacc.Bacc(None)

    fu_d = nc.declare_dram_parameter("fu", [SLAB, NCH, YP], BF16, isOutput=False)
    mk_d = nc.declare_dram_parameter("mk", [SLAB, YP], U8, isOutput=False)
    fxu_d = nc.declare_dram_parameter("fxu", [FX_PI, NCH * FX_F], BF16,
                                      isOutput=False)
    fxm_d = nc.declare_dram_parameter("fxm", [FX_PO, FX_W], U8, isOutput=False)
    fxo_d = nc.declare_dram_parameter("fxo", [FX_PO, NCO * FX_W], BF16,
                                      isOutput=True)
    shm_d = nc.declare_dram_parameter("shm", [128, SHM_COLS], BF16, isOutput=False)
    out_d = nc.declare_dram_parameter("out", [R, NCO, NY], BF16, isOutput=True)

    with tile.TileContext(nc) as tc, tc.tile_pool(name="cst", bufs=1) as cst:
        shm = cst.tile([128, SHM_COLS], BF16)
        nc.sync.dma_start(out=shm[:], in_=shm_d[:, :])
        with (
            tc.tile_pool(name="io", bufs=3) as io,
            tc.tile_pool(name="iof", bufs=3) as iof,
            tc.tile_pool(name="fsp", bufs=3) as fsp,
            tc.tile_pool(name="scr", bufs=1) as scr,
            tc.tile_pool(name="psA", bufs=3, space="PSUM") as psA,
            tc.tile_pool(name="psB", bufs=3, space="PSUM") as psB,
            tc.tile_pool(name="psR", bufs=1, space="PSUM") as psR,
        ):
            def stage1_main(tb, c0):
                """in-DMA, moments, fs assembly, stream, y-dir copies."""
                inM = io.tile([128, 3 * FW], BF16, tag="inM",
                              name=f"inM{tb}_{c0}")
                inF = iof.tile([128, 9 * FW], BF16, tag="inF",
                               name=f"inF{tb}_{c0}")
                mk8 = io.tile([128, FW], U8, tag="mk8",
                              name=f"mk8{tb}_{c0}")
                nc.sync.dma_start(
                    out=inM[:],
                    in_=fu_d[tb:tb + 128, 9:12, c0:c0 + FW])
                nc.sync.dma_start(
                    out=mk8[:], in_=mk_d[tb:tb + 128, c0:c0 + FW])
                nc.sync.dma_start(
                    out=inF[:],
                    in_=fu_d[tb:tb + 128, 0:9, c0:c0 + FW])
                iv = lambda c: (inF[:, c * FW:(c + 1) * FW] if c < 9
                                else inM[:, (c - 9) * FW:(c - 8) * FW])
                pl = _moments(nc, scr, 128, FW, iv)

                fs = fsp.tile([128, 9 * FW], BF16, tag="fs",
                              name=f"fs{tb}_{c0}")
                fsv = lambda i: fs[:, i * FW:(i + 1) * FW]
                _assemble_fs(nc, psA, shm, 128, FW,
                             [(0, 512)], pl, fsv,
                             f"t{tb}c{c0}")
                pq = psR.tile([128, 18], FP32, tag="slv",
                              name=f"t{tb}c{c0}slv")
                for i in range(9):
                    terms = ASPEC[i]
                    for k, (wc, pn) in enumerate(terms):
                        nc.tensor.matmul(
                            pq[:, i * 2:(i + 1) * 2],
                            shm[0:128, wc:wc + 128], pl[pn][:, 512:514],
                            start=(k == 0), stop=(k == len(terms) - 1))
                nc.scalar.copy(
                    fs[:].rearrange("p (d y) -> p d y", d=9)[:, :, 512:514],
                    pq[:].rearrange("p (d y) -> p d y", d=9))

                outT = io.tile([128, NCO * W], BF16, tag="outT",
                               name=f"outT{tb}_{c0}")
                ov = lambda i: outT[:, i * W:(i + 1) * W]
                for i in (1, 3, 5, 6, 7, 8):
                    wc = C_SP if EX[i] == 1 else C_SM
                    ysl = slice(1 - EY[i], 1 - EY[i] + W)
                    pf = psB.tile([128, W], FP32, tag="fnP",
                                  name=f"t{tb}c{c0}fn{i}")
                    nc.tensor.matmul(pf[:], shm[0:128, wc:wc + 128],
                                     fsv(i)[:, ysl])
                    nc.scalar.copy(outT[:, i * W: i * W + W], pf[:])
                return ("main", tb, c0, mk8, fs, outT)

            def stage2_main(st):
                """bounce-back, lift, out-DMA."""
                _, tb, c0, mk8, fs, outT = st
                fsv = lambda i: fs[:, i * FW:(i + 1) * FW]
                ov = lambda i: outT[:, i * W:(i + 1) * W]
                mk = mk8[:, 1:1 + W]
                f0view = fsv(0)
                nc.vector.tensor_copy(ov(2), fsv(2)[:, 0:W])
                nc.vector.tensor_copy(ov(4), fsv(4)[:, 2:2 + W])
                for i in range(1, 9):
                    nc.vector.copy_predicated(
                        ov(i), mk, fsv(OPP[i])[:, 1:1 + W])
                nc.sync.dma_start(
                    out=out_d[tb:tb + 126, 1:9, c0:c0 + W],
                    in_=outT[1:127, 1 * W:9 * W])
                inv = scr.tile([128, W], FP32, tag="inv", name="inv")
                pr = psR.tile([128, W], FP32, tag="rhoP",
                              name=f"t{tb}c{c0}rho")
                for k in range(9):
                    if k == 0:
                        rhs = f0view[:, 1:1 + W]
                    else:
                        rhs = outT[:, k * W: k * W + W]
                    nc.tensor.matmul(
                        pr[:], shm[0:128, C_I:C_I + 128], rhs,
                        start=(k == 0), stop=(k == 8))
                nc.scalar.copy(outT[:, 9 * W:10 * W], pr[:])
                nc.vector.reciprocal_approx_fast(inv[:], pr[:])
                _lift_finish(nc, scr, 128, W, ov, inv[:], f"t{tb}c{c0}")
                nc.sync.dma_start(
                    out=out_d[tb:tb + 126, 9:NCO, c0:c0 + W],
                    in_=outT[1:127, 9 * W:NCO * W])
                nc.sync.dma_start(
                    out=out_d[tb:tb + 126, 0, c0:c0 + W],
                    in_=f0view[1:127, 1:1 + W])

            def stage1_fx():
                fxin = io.tile([FX_PI, NCH * FX_F], BF16, tag="inM",
                               name="fxin")
                fxmk = scr.tile([FX_PO, FX_W], U8, tag="fxmk", name="fxmk")
                nc.sync.dma_start(out=fxin[:], in_=fxu_d[:, :])
                nc.sync.dma_start(out=fxmk[:], in_=fxm_d[:, :])
                fiv = lambda c: fxin[:, c * FX_F:(c + 1) * FX_F]
                fpl = _moments(nc, scr, FX_PI, FX_F, fiv)

                fxfs = fsp.tile([FX_PI, 9 * FX_F], BF16, tag="fs",
                                name="fxfs")
                ffsv = lambda i: fxfs[:, i * FX_F:(i + 1) * FX_F]
                _assemble_fs(nc, psA, shm, FX_PI, FX_F, [(0, FX_F)], fpl,
                             ffsv, "fx")

                fxout = io.tile([FX_PO, NCO * FX_W], BF16, tag="outT",
                                name="fxout")
                fov = lambda i: fxout[:, i * FX_W:(i + 1) * FX_W]
                for i in range(9):
                    wc = C_PX[EX[i]]
                    ysl = slice(1 - EY[i], 1 - EY[i] + FX_W)
                    pf = psB.tile([FX_PO, FX_W], FP32, tag="fnP",
                                  name=f"fxfn{i}")
                    nc.tensor.matmul(pf[:], shm[0:FX_PI, wc:wc + FX_PO],
                                     ffsv(i)[:, ysl])
                    nc.scalar.copy(fov(i), pf[:])
                return ("fx", fxin, fxfs, fxout, fxmk)

            def stage2_fx(st):
                _, fxin, fxfs, fxout, fxmk = st
                ffsv = lambda i: fxfs[:, i * FX_F:(i + 1) * FX_F]
                fov = lambda i: fxout[:, i * FX_W:(i + 1) * FX_W]
                for i in range(1, 9):
                    pq = psB.tile([FX_PO, FX_W], FP32, tag="fnP",
                                  name=f"fxbb{i}")
                    nc.tensor.matmul(pq[:],
                                     shm[0:FX_PI, C_PX[0]:C_PX[0] + FX_PO],
                                     ffsv(OPP[i])[:, 1:1 + FX_W])
                    nc.vector.copy_predicated(fov(i), fxmk[:], pq[:])
                pr = psR.tile([FX_PO, FX_W], FP32, tag="rhoP", name="fxrho")
                for k in range(9):
                    nc.tensor.matmul(pr[:], shm[0:FX_PO, C_I:C_I + FX_PO],
                                     fov(k), start=(k == 0), stop=(k == 8))
                nc.scalar.copy(fov(9), pr[:])
                fxinv = scr.tile([FX_PO, FX_W], FP32, tag="inv", name="fxinv")
                nc.vector.reciprocal_approx_fast(fxinv[:], pr[:])
                _lift_finish(nc, scr, FX_PO, FX_W, fov, fxinv[:], "fx")
                nc.sync.dma_start(out=fxo_d[:, :], in_=fxout[:])

            # software-pipelined emission: S1(k+1) before S2(k)
            specs = [(0, 0), (0, W), (0, 2 * W), (0, 3 * W), (130, 0), (130, W), None, (130, 2 * W), (130, 3 * W)]
            if DBG_NO_FIXUP:
                specs = [s for s in specs if s is not None]
            if DBG_ONE_CHUNK:
                specs = [(0, 0)]
            pend = []
            for sp in specs:
                st = stage1_fx() if sp is None else stage1_main(*sp)
                pend.append(st)
                if len(pend) > 2:
                    prev = pend.pop(0)
                    (stage2_fx if prev[0] == "fx" else stage2_main)(prev)
            for prev in pend:
                (stage2_fx if prev[0] == "fx" else stage2_main)(prev)

    nc.finalize()
    return nc


DBG_NO_FIXUP = False
DBG_ONE_CHUNK = False

_NC_CACHE = None


def _get_nc():
    global _NC_CACHE
    if _NC_CACHE is None:
        _NC_CACHE = _build_program()
    return _NC_CACHE


def _shm_np():
    import ml_dtypes
    m = np.zeros((128, SHM_COLS), np.float32)
    for k in range(128):
        m[k, C_I + k] = 1.0
        m[k, C_4I + k] = 4.0
        m[k, C_NI + k] = -1.0
        m[k, C_QI + k] = 0.25
        m[k, C_NQI + k] = -0.25
    for mm_ in range(1, 128):
        m[mm_ - 1, C_SP + mm_] = 1.0    # out m = in m-1  (ex=+1)
    for mm_ in range(0, 127):
        m[mm_ + 1, C_SM + mm_] = 1.0    # out m = in m+1  (ex=-1)
    # fixup perms: out q = sg*4+jj <- in k = sg*6 + (jj+1-ex)
    for ex in (1, 0, -1):
        base = C_PX[ex]
        for sg in range(FX_SEG):
            for jj in range(FX_NO):
                m[sg * FX_NR + jj + 1 - ex, base + sg * FX_NO + jj] = 1.0
    return m.astype(ml_dtypes.bfloat16)


def _host_planes(f, rho, u, _unused=None):
    import ml_dtypes
    planes = np.empty((NX, NCH, NY), np.float32)
    planes[:, 0:9] = np.moveaxis(f, -1, 1)
    planes[:, 0:9] *= FCOEF
    planes[:, 9] = (IT / 9.0) * rho
    planes[:, 10] = 3.0 * u[..., 0]
    planes[:, 11] = 3.0 * u[..., 1]
    return planes.astype(ml_dtypes.bfloat16)


def _pad_slab(pb, lo, hi):
    rows = np.take(pb, np.arange(lo - 1, hi + 1), axis=0, mode="wrap")
    return np.ascontiguousarray(
        np.concatenate([rows[:, :, -1:], rows, rows[:, :, :1]], axis=2))


def kernel(f, rho, u, obstacle_mask, _trace=False):
    f = np.asarray(f, dtype=np.float32)
    rho = np.asarray(rho, dtype=np.float32)
    u = np.asarray(u, dtype=np.float32)
    maskf = np.asarray(obstacle_mask).astype(np.float32)
    pb = _host_planes(f, rho, u, maskf)
    mk8 = np.asarray(obstacle_mask).astype(np.uint8)
    shm = _shm_np()
    in_maps = []
    for k in range(NCORES):
        rows = np.take(mk8, np.arange(k * R - 1, (k + 1) * R + 1), axis=0,
                       mode="wrap")
        mkslab = np.ascontiguousarray(
            np.concatenate([rows[:, -1:], rows, rows[:, :1]], axis=1))
        in_maps.append({"fu": _pad_slab(pb, k * R, (k + 1) * R),
                        "mk": mkslab, "shm": shm})

    for im in in_maps:
        slab = im["fu"]          # [SLAB, 12, YP] bf16
        mslab = im["mk"]         # [SLAB, YP] u8
        fxu = np.empty((FX_PI, NCH, FX_F), slab.dtype)
        fxm = np.empty((FX_PO, FX_W), np.uint8)
        for sg in range(FX_SEG):
            fxu[sg * FX_NR:(sg + 1) * FX_NR] = slab[
                FX_R0:FX_R0 + FX_NR, :, sg * FX_W:sg * FX_W + FX_F]
            fxm[sg * FX_NO:(sg + 1) * FX_NO] = mslab[
                FX_R0 + 1:FX_R0 + 1 + FX_NO,
                sg * FX_W + 1:sg * FX_W + 1 + FX_W]
        im["fxu"] = fxu.reshape(FX_PI, NCH * FX_F)
        im["fxm"] = fxm

    nc = _get_nc()
    res = run_bass_kernel_spmd(nc, in_maps, list(range(NCORES)),
                               trace=bool(_trace))
    outs = []
    for k in range(NCORES):
        o = np.array(res.results[k]["out"])  # [256, 12, 2048] bf16
        fxo = res.results[k]["fxo"].reshape(FX_PO, NCO, FX_W)
        for sg in range(FX_SEG):
            o[126:130, :, sg * FX_W:(sg + 1) * FX_W] = \
                fxo[sg * FX_NO:(sg + 1) * FX_NO]
        outs.append(o)
    out = np.concatenate(outs, axis=0)       # [2048, 12, 2048] bf16
    out = np.ascontiguousarray(out.transpose(0, 2, 1)).astype(np.float32)
    if _trace:
        return out, res
    return out

